# revision 2
# baseline (speedup 1.0000x reference)
"""Causal self-attention (B=2, S=2048, E=1024, H=16, D=64) on 8 trn2 NeuronCores.

Sharding: core c = (batch b = c // 4, head-group g = c % 4).  Each core computes
4 heads (one quarter of the 16) for one batch: projections q/k/v for its 256
output channels, then causal flash-style attention, writing out[b, :, 256g:256g+256].

Per-core kernel design (Bass/Tile):
  - Host pre-transposes hidden -> hT [E, S] (bf16) and weight slices -> wT [E, 256]
    (bf16) so all matmul contractions have K on partitions.
  - q/k projections (bf16, PSUM-accumulated over 8 E-chunks) produce qT/kT in
    [d, t] layout, copied to SBUF as float32r with scale 1/8 (q) and bias add.
  - v projection produces v in [t, d] layout; DVE copy splits heads into
    v_aug tiles [tk=128, 65*2] with a ones column per head (sum-of-exp trick).
  - scores^T tiles [tk=128, tq=512] per head via single f32r matmuls; the two
    heads of a pair run concurrently on PE row halves (K=64 each).
  - exp via ScalarE activation (attention-mask bias per tk partition), bf16 out.
  - causal masking: gpsimd affine_select zeroes the invalid region of
    diagonal-crossing tiles after exp.
  - attn @ v_aug accumulates unnormalized out^T [65, tq] in PSUM (bf16 matmuls);
    row 64 is the softmax denominator.
  - PE transpose [65,128] -> [128,65], then DVE reciprocal + tensor_scalar mul
    normalizes and writes [t, d] output tiles; DMA to DRAM.
"""

import numpy as np
import ml_dtypes

import concourse.bass as bass
import concourse.mybir as mybir
import concourse.tile as tile
from concourse import bacc
from concourse.bass_utils import run_bass_kernel_spmd

F32 = mybir.dt.float32
F32R = mybir.dt.float32r
BF16 = mybir.dt.bfloat16

B, S, E = 2, 2048, 1024
H, D = 16, 64
NCORES = 8
OC = 256          # output channels per core (4 heads)
NPAIR = 2         # head pairs per core
NT = S // 128     # 16 tk tiles
NT4 = S // 512    # 4 tq blocks

_cached_nc = None


def _build():
    nc = bacc.Bacc()

    hT = nc.declare_dram_parameter("hT", [E, S], BF16, isOutput=False)
    wqT = nc.declare_dram_parameter("wqT", [E, OC], BF16, isOutput=False)
    wkT = nc.declare_dram_parameter("wkT", [E, OC], BF16, isOutput=False)
    wvT = nc.declare_dram_parameter("wvT", [E, OC], BF16, isOutput=False)
    bqp = nc.declare_dram_parameter("bqp", [128, 2], F32, isOutput=False)
    bkp = nc.declare_dram_parameter("bkp", [128, 2], F32, isOutput=False)
    bvf = nc.declare_dram_parameter("bvf", [OC], F32, isOutput=False)
    mask_t = nc.declare_dram_parameter("mask_t", [128, NT], F32, isOutput=False)
    ident = nc.declare_dram_parameter("ident", [65, 65], F32, isOutput=False)
    out = nc.declare_dram_parameter("out", [S, OC], F32, isOutput=True)

    EXP = mybir.ActivationFunctionType.Exp
    ADD = mybir.AluOpType.add
    MULT = mybir.AluOpType.mult
    GE = mybir.AluOpType.is_ge

    with tile.TileContext(nc) as tc:
        with (
            tc.tile_pool(name="cst", bufs=1) as cst,
            tc.tile_pool(name="work", bufs=3) as work,
            tc.tile_pool(name="expp", bufs=6) as expp,
            tc.tile_pool(name="ps_small", bufs=2, space="PSUM") as ps_small,
            tc.tile_pool(name="ps_sc", bufs=2, space="PSUM") as ps_sc,
            tc.tile_pool(name="ps_out", bufs=2, space="PSUM") as ps_out,
        ):
            # ---- constants ----
            mask_sb = cst.tile([128, NT], F32, tag="mask")
            nc.sync.dma_start(out=mask_sb, in_=mask_t[:, :])
            ident_sb = cst.tile([65, 65], F32, tag="ident")
            nc.sync.dma_start(out=ident_sb, in_=ident[:, :])
            bq_sb = cst.tile([128, 2], F32, tag="bq")
            nc.sync.dma_start(out=bq_sb, in_=bqp[:, :])
            bk_sb = cst.tile([128, 2], F32, tag="bk")
            nc.sync.dma_start(out=bk_sb, in_=bkp[:, :])
            # bv broadcast to all partitions: [128, 256]
            bv_sb = cst.tile([128, OC], F32, tag="bv")
            nc.gpsimd.dma_start(out=bv_sb, in_=bvf[:].partition_broadcast(128))

            # ---- big resident inputs ----
            hT_sb = []
            for e in range(8):
                t = cst.tile([128, S], BF16, tag=f"hT{e}")
                nc.sync.dma_start(out=t, in_=hT[128 * e:128 * (e + 1), :])
                hT_sb.append(t)
            w_sb = {}
            for nm, src in (("q", wqT), ("k", wkT), ("v", wvT)):
                lst = []
                for e in range(8):
                    t = cst.tile([128, OC], BF16, tag=f"w{nm}{e}")
                    nc.sync.dma_start(out=t, in_=src[128 * e:128 * (e + 1), :])
                    lst.append(t)
                w_sb[nm] = lst

            # ---- persistent intermediates ----
            qT = [cst.tile([128, S], F32R, tag=f"qT{p}", name=f"qT{p}") for p in range(NPAIR)]
            kT = [cst.tile([128, S], F32R, tag=f"kT{p}", name=f"kT{p}") for p in range(NPAIR)]
            vaug = [[cst.tile([128, 130], BF16, tag=f"va{p}_{tt}", name=f"va{p}_{tt}")
                     for tt in range(NT)] for p in range(NPAIR)]
            outsb = [cst.tile([128, OC], F32, tag=f"o{tt}", name=f"o{tt}") for tt in range(NT)]

            for p in range(NPAIR):
                po = 128 * p  # column offset into the 256-wide slice

                # ---- q/k projections: psum [o=128, t=512] over 8 E-chunks ----
                for nm, dst, b_sb, scl in (("q", qT[p], bq_sb, 0.125),
                                           ("k", kT[p], bk_sb, None)):
                    for t4 in range(NT4):
                        ts = slice(512 * t4, 512 * (t4 + 1))
                        ps_qk = ps_small.tile([128, 512], F32, tag="sm")
                        for e in range(8):
                            nc.tensor.matmul(
                                ps_qk,
                                w_sb[nm][e][:, po:po + 128],
                                hT_sb[e][:, ts],
                                start=(e == 0), stop=(e == 7),
                            )
                        if scl is not None:
                            nc.vector.tensor_scalar(
                                out=dst[:, ts], in0=ps_qk,
                                scalar1=scl, scalar2=b_sb[:, p:p + 1],
                                op0=MULT, op1=ADD,
                            )
                        else:
                            nc.vector.tensor_scalar_add(
                                out=dst[:, ts], in0=ps_qk, scalar1=b_sb[:, p:p + 1],
                            )

                # ---- v projection: psum [t=128, o=128] over 8 E-chunks ----
                for tt in range(NT):
                    rs = slice(128 * tt, 128 * (tt + 1))
                    ps_v = ps_small.tile([128, 128], F32, tag="sm")
                    for e in range(8):
                        nc.tensor.matmul(
                            ps_v,
                            hT_sb[e][:, rs],
                            w_sb["v"][e][:, po:po + 128],
                            start=(e == 0), stop=(e == 7),
                        )
                    vt = vaug[p][tt]
                    # split the two heads into cols 0:64 and 65:129, adding bias
                    vt3 = vt.rearrange("a (h c) -> a h c", h=2)[:, :, 0:64]
                    ps3 = ps_v.rearrange("a (h c) -> a h c", h=2)
                    bv3 = bv_sb[:, po:po + 128].rearrange("a (h c) -> a h c", h=2)
                    nc.vector.tensor_add(vt3, ps3, bv3)
                    # ones columns at 64 and 129
                    nc.vector.memset(
                        vt.rearrange("a (h c) -> a h c", h=2)[:, :, 64:65], 1.0)

                # ---- attention ----
                for j in range(NT4):
                    qs = slice(512 * j, 512 * (j + 1))
                    out_A = ps_out.tile([65, 512], F32, tag="out")
                    out_B = ps_out.tile([65, 512], F32, tag="out")
                    ntk = 4 * (j + 1)
                    for i in range(ntk):
                        ks = slice(128 * i, 128 * (i + 1))
                        sc = ps_sc.tile([128, 1024], F32, tag="sc")
                        nc.tensor.matmul(sc[:, 0:512], kT[p][0:64, ks],
                                         qT[p][0:64, qs], start=True, stop=True)
                        nc.tensor.matmul(sc[:, 512:1024], kT[p][64:128, ks],
                                         qT[p][64:128, qs], start=True, stop=True)
                        ex = expp.tile([128, 1024], BF16, tag="exp")
                        nc.scalar.activation(out=ex, in_=sc, func=EXP,
                                             bias=mask_sb[:, i:i + 1], scale=1.0)
                        if i >= 4 * j:
                            # diagonal-crossing tile: zero where tq < tk, i.e.
                            # keep where (f - p - s) >= 0 with s = 128i - 512j
                            s_off = 128 * i - 512 * j
                            ex3 = ex.rearrange("a (h f) -> a h f", h=2)
                            nc.gpsimd.affine_select(
                                out=ex3, in_=ex3, compare_op=GE, fill=0.0,
                                base=-s_off, pattern=[[0, 2], [1, 512]],
                                channel_multiplier=-1,
                            )
                        nc.tensor.matmul(out_A, vaug[p][i][:, 0:65],
                                         ex[:, 0:512],
                                         start=(i == 0), stop=(i == ntk - 1))
                        nc.tensor.matmul(out_B, vaug[p][i][:, 65:130],
                                         ex[:, 512:1024],
                                         start=(i == 0), stop=(i == ntk - 1))

                    for h_loc, out_ps in ((0, out_A), (1, out_B)):
                        u = work.tile([65, 512], F32, tag="u")
                        nc.vector.tensor_copy(u, out_ps)
                        for s4 in range(4):
                            tp = ps_small.tile([128, 65], F32, tag="sm")
                            nc.tensor.transpose(tp, u[:, 128 * s4:128 * (s4 + 1)],
                                                ident_sb)
                            r = work.tile([128, 1], F32, tag="r")
                            nc.vector.reciprocal(r, tp[:, 64:65])
                            tt = 4 * j + s4
                            c0 = po + 64 * h_loc
                            nc.vector.tensor_scalar_mul(
                                outsb[tt][:, c0:c0 + 64], tp[:, 0:64], r)
                    if p == NPAIR - 1:
                        for s4 in range(4):
                            tt = 4 * j + s4
                            nc.sync.dma_start(
                                out=out[128 * tt:128 * (tt + 1), :],
                                in_=outsb[tt])

    nc.compile()
    return nc


def _get_nc():
    global _cached_nc
    if _cached_nc is None:
        _cached_nc = _build()
    return _cached_nc


def kernel(hidden_states, attention_mask, Wq, bq, Wk, bk, Wv, bv):
    hidden_states = np.asarray(hidden_states, dtype=np.float32)
    attention_mask = np.asarray(attention_mask, dtype=np.float32)
    Wq = np.asarray(Wq, dtype=np.float32)
    Wk = np.asarray(Wk, dtype=np.float32)
    Wv = np.asarray(Wv, dtype=np.float32)
    bq = np.asarray(bq, dtype=np.float32)
    bk = np.asarray(bk, dtype=np.float32)
    bv = np.asarray(bv, dtype=np.float32)

    bf = ml_dtypes.bfloat16
    ident = np.eye(65, dtype=np.float32)
    in_maps = []
    for c in range(NCORES):
        b, g = divmod(c, 4)
        cs = slice(OC * g, OC * (g + 1))
        in_maps.append({
            "hT": np.ascontiguousarray(hidden_states[b].T).astype(bf),
            "wqT": np.ascontiguousarray(Wq[cs, :].T).astype(bf),
            "wkT": np.ascontiguousarray(Wk[cs, :].T).astype(bf),
            "wvT": np.ascontiguousarray(Wv[cs, :].T).astype(bf),
            "bqp": np.ascontiguousarray(bq[cs].reshape(2, 128).T),
            "bkp": np.ascontiguousarray(bk[cs].reshape(2, 128).T),
            "bvf": np.ascontiguousarray(bv[cs]),
            "mask_t": np.ascontiguousarray(
                attention_mask[b, 0, 0, :].reshape(NT, 128).T),
            "ident": ident,
        })

    nc = _get_nc()
    res = run_bass_kernel_spmd(nc, in_maps, list(range(NCORES)))

    full = np.empty((B, S, H * D), dtype=np.float32)
    for c in range(NCORES):
        b, g = divmod(c, 4)
        full[b, :, OC * g:OC * (g + 1)] = res.results[c]["out"]
    return full


# revision 3
# speedup vs baseline: 1.0621x; 1.0621x over previous
"""Causal self-attention (B=2, S=2048, E=1024, H=16, D=64) on 8 trn2 NeuronCores.

Sharding: core c = (batch b = c // 4, head-group g = c % 4).  Each core computes
4 heads (one quarter of the 16) for one batch: projections q/k/v for its 256
output channels, then causal flash-style attention, writing out[b, :, 256g:256g+256].

Per-core kernel design (Bass/Tile):
  - Host pre-transposes hidden -> hT [E, S] (bf16) and weight slices -> wT [E, 256]
    (bf16) so all matmul contractions have K on partitions.
  - q/k projections (bf16, PSUM-accumulated over 8 E-chunks) produce qT/kT in
    [d, t] layout, copied to SBUF as float32r with scale 1/8 (q) and bias add.
  - v projection produces v in [t, d] layout; DVE copy splits heads into
    v_aug tiles [tk=128, 65*2] with a ones column per head (sum-of-exp trick).
  - scores^T tiles [tk=128, tq=512] per head via single f32r matmuls; the two
    heads of a pair run concurrently on PE row halves (K=64 each).
  - exp via ScalarE activation (attention-mask bias per tk partition), bf16 out.
  - causal masking: gpsimd affine_select zeroes the invalid region of
    diagonal-crossing tiles after exp.
  - attn @ v_aug accumulates unnormalized out^T [65, tq] in PSUM (bf16 matmuls);
    row 64 is the softmax denominator.
  - PE transpose [65,128] -> [128,65], then DVE reciprocal + tensor_scalar mul
    normalizes and writes [t, d] output tiles; DMA to DRAM.
"""

import numpy as np
import ml_dtypes

import concourse.bass as bass
import concourse.mybir as mybir
import concourse.tile as tile
from concourse import bacc
from concourse.bass_utils import run_bass_kernel_spmd

F32 = mybir.dt.float32
F32R = mybir.dt.float32r
BF16 = mybir.dt.bfloat16

B, S, E = 2, 2048, 1024
H, D = 16, 64
NCORES = 8
OC = 256          # output channels per core (4 heads)
NPAIR = 2         # head pairs per core
NT = S // 128     # 16 tk tiles
NT4 = S // 512    # 4 tq blocks

_cached_nc = None


def _build():
    nc = bacc.Bacc()

    hT = nc.declare_dram_parameter("hT", [E, S], BF16, isOutput=False)
    wqT = nc.declare_dram_parameter("wqT", [E, OC], BF16, isOutput=False)
    wkT = nc.declare_dram_parameter("wkT", [E, OC], BF16, isOutput=False)
    wvT = nc.declare_dram_parameter("wvT", [E, OC], BF16, isOutput=False)
    bqp = nc.declare_dram_parameter("bqp", [128, 2], F32, isOutput=False)
    bkp = nc.declare_dram_parameter("bkp", [128, 2], F32, isOutput=False)
    bvf = nc.declare_dram_parameter("bvf", [OC], F32, isOutput=False)
    mask_t = nc.declare_dram_parameter("mask_t", [128, NT], F32, isOutput=False)
    ident = nc.declare_dram_parameter("ident", [65, 65], F32, isOutput=False)
    out = nc.declare_dram_parameter("out", [S, OC], F32, isOutput=True)

    EXP = mybir.ActivationFunctionType.Exp
    ADD = mybir.AluOpType.add
    MULT = mybir.AluOpType.mult
    GE = mybir.AluOpType.is_ge

    with tile.TileContext(nc) as tc:
        with (
            tc.tile_pool(name="cst", bufs=1) as cst,
            tc.tile_pool(name="work", bufs=3) as work,
            tc.tile_pool(name="expp", bufs=6) as expp,
            tc.tile_pool(name="ps_small", bufs=2, space="PSUM") as ps_small,
            tc.tile_pool(name="ps_sc", bufs=2, space="PSUM") as ps_sc,
            tc.tile_pool(name="ps_out", bufs=2, space="PSUM") as ps_out,
        ):
            # ---- constants ----
            mask_sb = cst.tile([128, NT], F32, tag="mask")
            nc.sync.dma_start(out=mask_sb, in_=mask_t[:, :])
            ident_sb = cst.tile([65, 65], F32, tag="ident")
            nc.sync.dma_start(out=ident_sb, in_=ident[:, :])
            bq_sb = cst.tile([128, 2], F32, tag="bq")
            nc.sync.dma_start(out=bq_sb, in_=bqp[:, :])
            bk_sb = cst.tile([128, 2], F32, tag="bk")
            nc.sync.dma_start(out=bk_sb, in_=bkp[:, :])
            # bv broadcast to all partitions: [128, 256]
            bv_sb = cst.tile([128, OC], F32, tag="bv")
            nc.gpsimd.dma_start(out=bv_sb, in_=bvf[:].partition_broadcast(128))

            # ---- big resident inputs ----
            hT_sb = []
            for e in range(8):
                t = cst.tile([128, S], BF16, tag=f"hT{e}")
                nc.sync.dma_start(out=t, in_=hT[128 * e:128 * (e + 1), :])
                hT_sb.append(t)
            w_sb = {}
            for nm, src in (("q", wqT), ("k", wkT), ("v", wvT)):
                lst = []
                for e in range(8):
                    t = cst.tile([128, OC], BF16, tag=f"w{nm}{e}")
                    nc.sync.dma_start(out=t, in_=src[128 * e:128 * (e + 1), :])
                    lst.append(t)
                w_sb[nm] = lst

            # ---- persistent intermediates ----
            qT = [cst.tile([128, S], BF16, tag=f"qT{p}", name=f"qT{p}") for p in range(NPAIR)]
            kT = [cst.tile([128, S], BF16, tag=f"kT{p}", name=f"kT{p}") for p in range(NPAIR)]
            vaug = [[cst.tile([128, 130], BF16, tag=f"va{p}_{tt}", name=f"va{p}_{tt}")
                     for tt in range(NT)] for p in range(NPAIR)]
            outsb = [cst.tile([128, OC], F32, tag=f"o{tt}", name=f"o{tt}") for tt in range(NT)]

            # ---- projections: q/k for both pairs ----
            for p in range(NPAIR):
                po = 128 * p
                for nm, dst, b_sb, scl in (("q", qT[p], bq_sb, 0.125),
                                           ("k", kT[p], bk_sb, None)):
                    for t4 in range(NT4):
                        ts = slice(512 * t4, 512 * (t4 + 1))
                        ps_qk = ps_small.tile([128, 512], F32, tag="sm", name="ps_qk")
                        for e in range(8):
                            nc.tensor.matmul(
                                ps_qk,
                                w_sb[nm][e][:, po:po + 128],
                                hT_sb[e][:, ts],
                                start=(e == 0), stop=(e == 7),
                            )
                        if scl is not None:
                            nc.vector.tensor_scalar(
                                out=dst[:, ts], in0=ps_qk,
                                scalar1=scl, scalar2=b_sb[:, p:p + 1],
                                op0=MULT, op1=ADD,
                            )
                        else:
                            nc.vector.tensor_scalar_add(
                                out=dst[:, ts], in0=ps_qk, scalar1=b_sb[:, p:p + 1],
                            )

            # ---- v projection, both pairs at once: psum [t=128, o=256] ----
            for tt in range(NT):
                rs = slice(128 * tt, 128 * (tt + 1))
                ps_v = ps_small.tile([128, OC], F32, tag="sm", name="ps_v")
                for e in range(8):
                    nc.tensor.matmul(
                        ps_v,
                        hT_sb[e][:, rs],
                        w_sb["v"][e][:, :],
                        start=(e == 0), stop=(e == 7),
                    )
                for p in range(NPAIR):
                    po = 128 * p
                    vt = vaug[p][tt]
                    vt3 = vt.rearrange("a (h c) -> a h c", h=2)[:, :, 0:64]
                    ps3 = ps_v[:, po:po + 128].rearrange("a (h c) -> a h c", h=2)
                    bv3 = bv_sb[:, po:po + 128].rearrange("a (h c) -> a h c", h=2)
                    nc.vector.tensor_add(vt3, ps3, bv3)
                    nc.vector.memset(
                        vt.rearrange("a (h c) -> a h c", h=2)[:, :, 64:65], 1.0)

            # ---- attention, per pair, tq blocks in descending size order ----
            for p in range(NPAIR):
                po = 128 * p
                for j in reversed(range(NT4)):
                    qs = slice(512 * j, 512 * (j + 1))
                    out_A = ps_out.tile([65, 512], F32, tag="out", name="out_A")
                    out_B = ps_out.tile([65, 512], F32, tag="out", name="out_B")
                    ntk = 4 * (j + 1)
                    for i in range(ntk):
                        ks = slice(128 * i, 128 * (i + 1))
                        sc = ps_sc.tile([128, 1024], F32, tag="sc", name="sc")
                        nc.tensor.matmul(sc[:, 0:512], kT[p][0:64, ks],
                                         qT[p][0:64, qs], start=True, stop=True,
                                         tile_position=(0, 0))
                        nc.tensor.matmul(sc[:, 512:1024], kT[p][64:128, ks],
                                         qT[p][64:128, qs], start=True, stop=True,
                                         tile_position=(64, 0))
                        ex = expp.tile([128, 1024], BF16, tag="exp", name="ex")
                        nc.scalar.activation(out=ex, in_=sc, func=EXP,
                                             bias=mask_sb[:, i:i + 1], scale=1.0)
                        if i >= 4 * j:
                            s_off = 128 * i - 512 * j
                            ex3 = ex.rearrange("a (h f) -> a h f", h=2)
                            nc.gpsimd.affine_select(
                                out=ex3, in_=ex3, compare_op=GE, fill=0.0,
                                base=-s_off, pattern=[[0, 2], [1, 512]],
                                channel_multiplier=-1,
                            )
                        nc.tensor.matmul(out_A, vaug[p][i][:, 0:65],
                                         ex[:, 0:512],
                                         start=(i == 0), stop=(i == ntk - 1))
                        nc.tensor.matmul(out_B, vaug[p][i][:, 65:130],
                                         ex[:, 512:1024],
                                         start=(i == 0), stop=(i == ntk - 1))

                    for h_loc, out_ps in ((0, out_A), (1, out_B)):
                        u = work.tile([65, 512], F32, tag="u", name="u")
                        nc.vector.tensor_copy(u, out_ps)
                        for s4 in range(4):
                            tp = ps_small.tile([128, 65], F32, tag="sm", name="tp")
                            nc.tensor.transpose(tp, u[:, 128 * s4:128 * (s4 + 1)],
                                                ident_sb)
                            r = work.tile([128, 1], F32, tag="r", name="r")
                            nc.vector.reciprocal(r, tp[:, 64:65])
                            tt = 4 * j + s4
                            c0 = po + 64 * h_loc
                            nc.vector.tensor_scalar_mul(
                                outsb[tt][:, c0:c0 + 64], tp[:, 0:64], r)
                    if p == NPAIR - 1:
                        for s4 in range(4):
                            tt = 4 * j + s4
                            nc.sync.dma_start(
                                out=out[128 * tt:128 * (tt + 1), :],
                                in_=outsb[tt])

    nc.compile()
    return nc


def _get_nc():
    global _cached_nc
    if _cached_nc is None:
        _cached_nc = _build()
    return _cached_nc


def kernel(hidden_states, attention_mask, Wq, bq, Wk, bk, Wv, bv):
    hidden_states = np.asarray(hidden_states, dtype=np.float32)
    attention_mask = np.asarray(attention_mask, dtype=np.float32)
    Wq = np.asarray(Wq, dtype=np.float32)
    Wk = np.asarray(Wk, dtype=np.float32)
    Wv = np.asarray(Wv, dtype=np.float32)
    bq = np.asarray(bq, dtype=np.float32)
    bk = np.asarray(bk, dtype=np.float32)
    bv = np.asarray(bv, dtype=np.float32)

    bf = ml_dtypes.bfloat16
    ident = np.eye(65, dtype=np.float32)
    in_maps = []
    for c in range(NCORES):
        b, g = divmod(c, 4)
        cs = slice(OC * g, OC * (g + 1))
        in_maps.append({
            "hT": np.ascontiguousarray(hidden_states[b].T).astype(bf),
            "wqT": np.ascontiguousarray(Wq[cs, :].T).astype(bf),
            "wkT": np.ascontiguousarray(Wk[cs, :].T).astype(bf),
            "wvT": np.ascontiguousarray(Wv[cs, :].T).astype(bf),
            "bqp": np.ascontiguousarray(bq[cs].reshape(2, 128).T),
            "bkp": np.ascontiguousarray(bk[cs].reshape(2, 128).T),
            "bvf": np.ascontiguousarray(bv[cs]),
            "mask_t": np.ascontiguousarray(
                attention_mask[b, 0, 0, :].reshape(NT, 128).T),
            "ident": ident,
        })

    nc = _get_nc()
    res = run_bass_kernel_spmd(nc, in_maps, list(range(NCORES)))

    full = np.empty((B, S, H * D), dtype=np.float32)
    for c in range(NCORES):
        b, g = divmod(c, 4)
        full[b, :, OC * g:OC * (g + 1)] = res.results[c]["out"]
    return full
